# Initial kernel scaffold
#
"""Segment-mean aggregation (MeanAggregation) on 8 Trainium2 NeuronCores.

Strategy
--------
H_v is [524288, 512] f32 with contiguous per-graph node blocks (sizes sum
exactly to 524288).  We shard the NODE stream uniformly: core d gets nodes
[65536*d, 65536*(d+1)) -- a pure reshape view, no padding.

On each core the node stream is processed in 32 "epochs" of 2048 nodes
(16 chunks x 128 nodes).  For every 128-node chunk we build a [128 nodes,
128 graphs] one-hot matrix on-device (DVE is_equal of an iota row against
the node's *epoch-local* segment id, which the host packs into a small f32
side input), then accumulate 16 matmuls  onehot.T @ H_chunk  into one PSUM
bank [128, 512].  Each epoch's bank holds the partial sums for the <=128
graphs touched by its 2048 nodes.  The host scatter-adds the 8x32 epoch
windows into the [16384, 512] output (graphs straddling chunk/epoch/core
boundaries get partial sums from several windows; addition merges them),
then divides by max(sizes, 1).

A 2048-node window spans ~64 graphs (avg graph size 32) so the <=128-graph
window always holds in practice; the host verifies it and falls back to a
numpy path if an adversarial `sizes` violates it.
"""

import sys

if "/opt/trn_rl_repo" not in sys.path:
    sys.path.insert(0, "/opt/trn_rl_repo")

import contextlib

import numpy as np

import concourse.bacc as bacc
import concourse.mybir as mybir
import concourse.tile as tile
from concourse import bass_utils

N_NODES = 524288
N_GRAPHS = 16384
D = 512
N_CORES = 8
P = 128

NODES_PER_CORE = N_NODES // N_CORES          # 65536
CHUNKS_PER_CORE = NODES_PER_CORE // P        # 512
EPOCH_CHUNKS = 16                            # chunks per PSUM accumulation group
EPOCH_NODES = EPOCH_CHUNKS * P               # 2048
EPOCHS = CHUNKS_PER_CORE // EPOCH_CHUNKS     # 32

H_BUFS = 4
TRACE = False          # set by test harness for profiling runs
LAST_RUN = {}          # exec_time_ns etc. from the last device run

_compiled_nc = None


def _build():
    """Build + compile the per-core Bass program (identical on all cores)."""
    nc = bacc.Bacc(
        "TRN2", target_bir_lowering=False, debug=False, num_devices=N_CORES
    )
    H = nc.dram_tensor(
        "H", [NODES_PER_CORE, D], mybir.dt.float32, kind="ExternalInput"
    ).ap()
    seg = nc.dram_tensor(
        "seg", [P, CHUNKS_PER_CORE], mybir.dt.float32, kind="ExternalInput"
    ).ap()
    out = nc.dram_tensor(
        "out", [EPOCHS, P, D], mybir.dt.float32, kind="ExternalOutput"
    ).ap()

    # epoch-major view of the node stream: [epoch, partition, chunk*feature]
    H_r = H.rearrange("(e c p) d -> e p (c d)", e=EPOCHS, c=EPOCH_CHUNKS, p=P)

    with tile.TileContext(nc) as tc, contextlib.ExitStack() as ctx:
        const_pool = ctx.enter_context(tc.tile_pool(name="const", bufs=1))
        seg_pool = ctx.enter_context(tc.tile_pool(name="segp", bufs=1))
        h_pool = ctx.enter_context(tc.tile_pool(name="h", bufs=H_BUFS))
        oh_pool = ctx.enter_context(tc.tile_pool(name="oh", bufs=3))
        out_pool = ctx.enter_context(tc.tile_pool(name="o", bufs=4))
        psum_pool = ctx.enter_context(tc.tile_pool(name="ps", bufs=8, space="PSUM"))

        # iota row 0..127 on every partition, as f32 (values are exact)
        iota_i = const_pool.tile([P, P], mybir.dt.int32)
        nc.gpsimd.iota(iota_i[:], [[1, P]], channel_multiplier=0)
        iota_f = const_pool.tile([P, P], mybir.dt.float32)
        nc.vector.tensor_copy(iota_f[:], iota_i[:])

        seg_t = seg_pool.tile([P, CHUNKS_PER_CORE], mybir.dt.float32)
        nc.sync.dma_start(seg_t[:], seg[:])

        for e in range(EPOCHS):
            h_t = h_pool.tile([P, EPOCH_CHUNKS * D], mybir.dt.float32)
            nc.sync.dma_start(h_t[:], H_r[e])

            oh_t = oh_pool.tile([P, EPOCH_CHUNKS * P], mybir.dt.float32)
            for c in range(EPOCH_CHUNKS):
                col = e * EPOCH_CHUNKS + c
                nc.vector.tensor_scalar(
                    out=oh_t[:, c * P : (c + 1) * P],
                    in0=iota_f[:],
                    scalar1=seg_t[:, col : col + 1],
                    scalar2=None,
                    op0=mybir.AluOpType.is_equal,
                )

            psum_t = psum_pool.tile([P, D], mybir.dt.float32, space="PSUM")
            for c in range(EPOCH_CHUNKS):
                nc.tensor.matmul(
                    out=psum_t[:],
                    lhsT=oh_t[:, c * P : (c + 1) * P],
                    rhs=h_t[:, c * D : (c + 1) * D],
                    start=(c == 0),
                    stop=(c == EPOCH_CHUNKS - 1),
                )

            o_t = out_pool.tile([P, D], mybir.dt.float32)
            nc.vector.tensor_copy(o_t[:], psum_t[:])
            nc.sync.dma_start(out[e], o_t[:])

    nc.compile()
    return nc


def _get_compiled():
    global _compiled_nc
    if _compiled_nc is None:
        _compiled_nc = _build()
    return _compiled_nc


def _seg_ids_like_reference(sizes):
    """seg id per node row, matching jnp.repeat(..., total_repeat_length=N)."""
    reps = np.clip(sizes.astype(np.int64), 0, None)
    seg = np.repeat(np.arange(N_GRAPHS, dtype=np.int64), reps)
    if seg.shape[0] >= N_NODES:
        return seg[:N_NODES]
    pad_val = seg[-1] if seg.shape[0] else 0
    return np.concatenate(
        [seg, np.full(N_NODES - seg.shape[0], pad_val, dtype=np.int64)]
    )


def _numpy_fallback(H, seg_ids, sizes):
    # exact segment sum for sorted seg_ids (np.repeat output is sorted)
    uniq, starts = np.unique(seg_ids, return_index=True)
    part = np.add.reduceat(H, starts, axis=0)
    sums = np.zeros((N_GRAPHS, D), np.float32)
    sums[uniq] = part
    denom = np.maximum(sizes, 1).astype(np.float32)[:, None]
    return sums / denom


def kernel(H_v, sizes):
    H = np.asarray(H_v, dtype=np.float32)
    sz = np.asarray(sizes, dtype=np.int32)
    seg_ids = _seg_ids_like_reference(sz)

    seg4 = seg_ids.reshape(N_CORES, EPOCHS, EPOCH_CHUNKS, P)
    base = seg4[:, :, 0, 0].copy()                     # [8, 32] epoch graph base
    local = seg4 - base[:, :, None, None]
    if local.min() < 0 or local.max() >= P:
        return _numpy_fallback(H, seg_ids, sz)

    # seg_packed[core][p, e*16+c] = epoch-local seg id of node (e,c,p)
    seg_packed = np.ascontiguousarray(
        local.transpose(0, 3, 1, 2).reshape(N_CORES, P, CHUNKS_PER_CORE)
    ).astype(np.float32)

    nc = _get_compiled()
    H8 = H.reshape(N_CORES, NODES_PER_CORE, D)
    in_maps = [{"H": H8[c], "seg": seg_packed[c]} for c in range(N_CORES)]
    res = bass_utils.run_bass_kernel_spmd(
        nc, in_maps, core_ids=list(range(N_CORES)), trace=TRACE
    )
    LAST_RUN["exec_time_ns"] = res.exec_time_ns
    LAST_RUN["mean_exec_time_ns"] = res.mean_exec_time_ns
    LAST_RUN["trace"] = res.instructions_and_trace

    out = np.zeros((N_GRAPHS + P, D), np.float32)      # +P: windows may overhang
    for c in range(N_CORES):
        st = res.results[c]["out"]                     # [32, 128, 512]
        for e in range(EPOCHS):
            b = base[c, e]
            out[b : b + P] += st[e]
    denom = np.maximum(sz, 1).astype(np.float32)[:, None]
    return out[:N_GRAPHS] / denom


# revision 8
# speedup vs baseline: 6.3171x; 6.3171x over previous
"""Segment-mean aggregation (MeanAggregation) on 8 Trainium2 NeuronCores.

Strategy
--------
H_v is [524288, 512] f32 with contiguous per-graph node blocks (sizes sum
exactly to 524288).  We shard the NODE stream uniformly: core d gets nodes
[65536*d, 65536*(d+1)) -- a pure reshape view, no padding.

On each core the node stream is processed in 32 "epochs" of 2048 nodes
(16 chunks x 128 nodes).  For every 128-node chunk we build a [128 nodes,
128 graphs] one-hot matrix on-device (DVE is_equal of an iota row against
the node's *epoch-local* segment id, which the host packs into a small f32
side input), then accumulate 16 matmuls  onehot.T @ H_chunk  into one PSUM
bank [128, 512].  Each epoch's bank holds the partial sums for the <=128
graphs touched by its 2048 nodes.  The host scatter-adds the 8x32 epoch
windows into the [16384, 512] output (graphs straddling chunk/epoch/core
boundaries get partial sums from several windows; addition merges them),
then divides by max(sizes, 1).

A 2048-node window spans ~64 graphs (avg graph size 32) so the <=128-graph
window always holds in practice; the host verifies it and falls back to a
numpy path if an adversarial `sizes` violates it.
"""

import sys

if "/opt/trn_rl_repo" not in sys.path:
    sys.path.insert(0, "/opt/trn_rl_repo")

import contextlib

import numpy as np

import concourse.bacc as bacc
import concourse.mybir as mybir
import concourse.tile as tile
from concourse import bass_utils

N_NODES = 524288
N_GRAPHS = 16384
D = 512
N_CORES = 8
P = 128

NODES_PER_CORE = N_NODES // N_CORES          # 65536
CHUNKS_PER_CORE = NODES_PER_CORE // P        # 512
EPOCH_CHUNKS = 16                            # chunks per PSUM accumulation group
EPOCH_NODES = EPOCH_CHUNKS * P               # 2048
EPOCHS = CHUNKS_PER_CORE // EPOCH_CHUNKS     # 32

H_BUFS = 4
TRACE = False          # set by test harness for profiling runs
LAST_RUN = {}          # exec_time_ns etc. from the last device run

_compiled_nc = None


def _build(repeats=1):
    """Build + compile the per-core Bass program (identical on all cores).

    H arrives split as fp16 hi/lo (host-side: hi = fp16(x), lo = fp16(x-hi)):
    fp32 matmuls run at 1/4 PE rate, fp16 at full rate, so two fp16 matmuls
    into the same fp32 PSUM group reconstruct the fp32 segment-sum exactly
    (products of fp16 are exact in fp32) at half the fp32 PE cost -- and the
    same DMA byte count.

    repeats > 1 re-runs the whole pipeline (for slope-based HW timing).
    """
    nc = bacc.Bacc(
        "TRN2", target_bir_lowering=False, debug=False, num_devices=N_CORES
    )
    Hhi = nc.dram_tensor(
        "Hhi", [NODES_PER_CORE, D], mybir.dt.float16, kind="ExternalInput"
    ).ap()
    Hlo = nc.dram_tensor(
        "Hlo", [NODES_PER_CORE, D], mybir.dt.float16, kind="ExternalInput"
    ).ap()
    seg = nc.dram_tensor(
        "seg", [P, CHUNKS_PER_CORE], mybir.dt.float32, kind="ExternalInput"
    ).ap()
    out = nc.dram_tensor(
        "out", [EPOCHS, P, D], mybir.dt.float32, kind="ExternalOutput"
    ).ap()

    # epoch-major view of the node stream: [epoch, partition, chunk, feature]
    Hhi_r = Hhi.rearrange("(e c p) d -> e p c d", e=EPOCHS, c=EPOCH_CHUNKS, p=P)
    Hlo_r = Hlo.rearrange("(e c p) d -> e p c d", e=EPOCHS, c=EPOCH_CHUNKS, p=P)

    with tile.TileContext(nc) as tc, contextlib.ExitStack() as ctx:
        const_pool = ctx.enter_context(tc.tile_pool(name="const", bufs=1))
        seg_pool = ctx.enter_context(tc.tile_pool(name="segp", bufs=1))
        h_pool = ctx.enter_context(tc.tile_pool(name="h", bufs=H_BUFS))
        oh_pool = ctx.enter_context(tc.tile_pool(name="oh", bufs=3))
        out_pool = ctx.enter_context(tc.tile_pool(name="o", bufs=4))
        psum_pool = ctx.enter_context(tc.tile_pool(name="ps", bufs=8, space="PSUM"))

        # iota row 0..127 on every partition, as fp16 (values <=127 are exact)
        iota_i = const_pool.tile([P, P], mybir.dt.int32)
        nc.gpsimd.iota(iota_i[:], [[1, P]], channel_multiplier=0)
        iota_f = const_pool.tile([P, P], mybir.dt.float16)
        nc.vector.tensor_copy(iota_f[:], iota_i[:])

        seg_t = seg_pool.tile([P, CHUNKS_PER_CORE], mybir.dt.float32)
        nc.sync.dma_start(seg_t[:], seg[:])

        for e in [e for _ in range(repeats) for e in range(EPOCHS)]:
            hi_t = h_pool.tile([P, EPOCH_CHUNKS * D], mybir.dt.float16, tag="hi")
            nc.sync.dma_start(
                hi_t[:].rearrange("p (c d) -> p c d", c=EPOCH_CHUNKS), Hhi_r[e]
            )
            lo_t = h_pool.tile([P, EPOCH_CHUNKS * D], mybir.dt.float16, tag="lo")
            nc.sync.dma_start(
                lo_t[:].rearrange("p (c d) -> p c d", c=EPOCH_CHUNKS), Hlo_r[e]
            )

            oh_t = oh_pool.tile([P, EPOCH_CHUNKS * P], mybir.dt.float16)
            for c in range(EPOCH_CHUNKS):
                col = e * EPOCH_CHUNKS + c
                nc.vector.tensor_scalar(
                    out=oh_t[:, c * P : (c + 1) * P],
                    in0=iota_f[:],
                    scalar1=seg_t[:, col : col + 1],
                    scalar2=None,
                    op0=mybir.AluOpType.is_equal,
                )

            psum_t = psum_pool.tile([P, D], mybir.dt.float32, space="PSUM")
            n_mm = 2 * EPOCH_CHUNKS
            for c in range(EPOCH_CHUNKS):
                for k, src in ((0, hi_t), (1, lo_t)):
                    nc.tensor.matmul(
                        out=psum_t[:],
                        lhsT=oh_t[:, c * P : (c + 1) * P],
                        rhs=src[:, c * D : (c + 1) * D],
                        start=(c == 0 and k == 0),
                        stop=(c == EPOCH_CHUNKS - 1 and k == 1),
                    )

            o_t = out_pool.tile([P, D], mybir.dt.float32)
            nc.vector.tensor_copy(o_t[:], psum_t[:])
            nc.sync.dma_start(out[e], o_t[:])

    nc.compile()
    return nc


def _get_compiled():
    global _compiled_nc
    if _compiled_nc is None:
        _compiled_nc = _build()
    return _compiled_nc


def _seg_ids_like_reference(sizes):
    """seg id per node row, matching jnp.repeat(..., total_repeat_length=N)."""
    reps = np.clip(sizes.astype(np.int64), 0, None)
    seg = np.repeat(np.arange(N_GRAPHS, dtype=np.int64), reps)
    if seg.shape[0] >= N_NODES:
        return seg[:N_NODES]
    pad_val = seg[-1] if seg.shape[0] else 0
    return np.concatenate(
        [seg, np.full(N_NODES - seg.shape[0], pad_val, dtype=np.int64)]
    )


def _numpy_fallback(H, seg_ids, sizes):
    # exact segment sum for sorted seg_ids (np.repeat output is sorted)
    uniq, starts = np.unique(seg_ids, return_index=True)
    part = np.add.reduceat(H, starts, axis=0)
    sums = np.zeros((N_GRAPHS, D), np.float32)
    sums[uniq] = part
    denom = np.maximum(sizes, 1).astype(np.float32)[:, None]
    return sums / denom


def kernel(H_v, sizes):
    H = np.asarray(H_v, dtype=np.float32)
    sz = np.asarray(sizes, dtype=np.int32)
    seg_ids = _seg_ids_like_reference(sz)

    seg4 = seg_ids.reshape(N_CORES, EPOCHS, EPOCH_CHUNKS, P)
    base = seg4[:, :, 0, 0].copy()                     # [8, 32] epoch graph base
    local = seg4 - base[:, :, None, None]
    if local.min() < 0 or local.max() >= P:
        return _numpy_fallback(H, seg_ids, sz)

    # seg_packed[core][p, e*16+c] = epoch-local seg id of node (e,c,p)
    seg_packed = np.ascontiguousarray(
        local.transpose(0, 3, 1, 2).reshape(N_CORES, P, CHUNKS_PER_CORE)
    ).astype(np.float32)

    # fp16 hi/lo split: hi + lo == H to ~2^-23 relative (fp32 quality)
    H_hi = H.astype(np.float16)
    H_lo = (H - H_hi.astype(np.float32)).astype(np.float16)

    nc = _get_compiled()
    hi8 = H_hi.reshape(N_CORES, NODES_PER_CORE, D)
    lo8 = H_lo.reshape(N_CORES, NODES_PER_CORE, D)
    in_maps = [
        {"Hhi": hi8[c], "Hlo": lo8[c], "seg": seg_packed[c]}
        for c in range(N_CORES)
    ]
    res = bass_utils.run_bass_kernel_spmd(
        nc, in_maps, core_ids=list(range(N_CORES)), trace=TRACE
    )
    LAST_RUN["exec_time_ns"] = res.exec_time_ns
    LAST_RUN["mean_exec_time_ns"] = res.mean_exec_time_ns
    LAST_RUN["trace"] = res.instructions_and_trace

    out = np.zeros((N_GRAPHS + P, D), np.float32)      # +P: windows may overhang
    for c in range(N_CORES):
        st = res.results[c]["out"]                     # [32, 128, 512]
        for e in range(EPOCHS):
            b = base[c, e]
            out[b : b + P] += st[e]
    denom = np.maximum(sz, 1).astype(np.float32)[:, None]
    return out[:N_GRAPHS] / denom
